# revision 1
# baseline (speedup 1.0000x reference)
"""CapsuleLayer (dynamic routing, 3 iters) on 8 TRN2 NeuronCores.

Strategy: shard the num_routes axis S=512 into 64 s-values per core.
Each core:
  phase 1: u_hat[b,c,s_loc,o] = x[b,s,:] @ W[c,s,:,:]  (PE, float32r)
           streamed from HBM (W is 64 MiB/core — the kernel is DMA-bound),
           u_hat kept in SBUF; running sum over local s accumulated for iter 0.
  phase 2: routing iterations on DVE/ACT. softmax over capsules is local
           (c lives on the free axis); only s_j = sum_s c_ij*u_hat needs a
           cross-core AllReduce ([B,C,dout] = 128 KiB) — 3 per kernel.

Layout: partition p = 32*j + b where j = s%4, b = batch; free dims (g=s//4, c, o).
"""
import numpy as np

import concourse.bass as bass
import concourse.mybir as mybir
import concourse.tile as tile
from concourse import bacc
from concourse.bass_utils import run_bass_kernel_spmd
from concourse.masks import make_identity

B, S, C, DIN, DOUT = 32, 512, 16, 256, 64
NCORES = 8
S_LOC = S // NCORES          # 64
NG = S_LOC // 4              # 16 groups of 4 s-values
CO = C * DOUT                # 1024
KI = DIN // 128              # 2 contraction chunks
F32 = mybir.dt.float32
F32R = mybir.dt.float32r
F16 = mybir.dt.float16
AX = mybir.AxisListType
ALU = mybir.AluOpType
ACTF = mybir.ActivationFunctionType

_CACHE = {}


def _register_mul_cumsum():
    """out[p, :] = running cumsum of in0*in1 along the free stream.

    Registered at runtime (dve_ops.py is read-only here); same mechanism as
    the production ops — the per-NEFF DVE table is generated from OPS by
    name at compile time."""
    from concourse import dve_ops
    from concourse.dve_spec import Spec, Src0, Src1, AluOp, scan, lower as dve_lower
    from concourse.dve_uop import DveOpSpec

    name = "MUL_CUMSUM_ANT"
    for op in dve_ops.OPS:
        if op.name == name:
            return op

    def _ref(in0, in1, s0, s1, imm2):
        prod = (np.asarray(in0, np.float32) * np.asarray(in1, np.float32)).astype(
            np.float32
        )
        flat = prod.reshape(prod.shape[0], -1)
        return np.cumsum(flat, axis=1, dtype=np.float32).reshape(prod.shape)

    spec = Spec(body=scan(AluOp.ADD, Src0 * Src1), reference=_ref)
    row = dve_ops._CUSTOM_DVE_ROW_BASE + len(dve_ops.OPS)
    assert row < 0x20
    dve_ops._SUB_OPCODE_FOR_NAME[name] = row
    shas = {}
    for ver in ("v3", "v4"):
        uops = dve_lower(spec, ver=ver)
        shas[ver] = DveOpSpec(name=name, opcode=row, uops=uops, rd1_en=True).sha(ver)
    op = dve_ops.DveOp(name, spec, subdim=False, uops_sha=shas)
    dve_ops.OPS.append(op)
    dve_ops.CUSTOM_DVE_SPECS[name] = spec
    return op


MUL_CUMSUM = _register_mul_cumsum()


def _build(sim_local=False, skip_routing=False, wbufs=3, dma_spread=0):
    nc = bacc.Bacc("TRN2", target_bir_lowering=False, debug=False, num_devices=NCORES)
    # Host pre-transposed inputs (per-core shards):
    #   xT: [S_LOC, DIN, B], wT: [S_LOC, DIN, C*DOUT]
    xT_ext = nc.declare_dram_parameter("xT", [128, KI, S_LOC, B], F16, isOutput=False)
    wT_ext = nc.declare_dram_parameter("wT", [S_LOC, DIN, CO], F16, isOutput=False)
    out_ext = nc.declare_dram_parameter("out", [B, CO], F32, isOutput=True)

    cc_in = [nc.dram_tensor(f"cc_in{k}", [B, CO // 2], F32) for k in range(4)]
    cc_out = [
        nc.dram_tensor(f"cc_out{k}", [B, CO // 2], F32, addr_space="Shared")
        for k in range(4)
    ]
    groups = [list(range(NCORES))]

    with tile.TileContext(nc) as tc:
        with tc.tile_pool(name="persist", bufs=1) as pp:
            # ---------------- phase 1: u_hat ----------------
            U = pp.tile([128, NG, C, DOUT], F32)      # u_hat, 64 KiB/part
            ACC = pp.tile([128, CO], F32)             # sum over local s (for iter 0)
            XK = pp.tile([128, KI, S_LOC, B], F16)   # x, stationary operands
            nc.sync.dma_start(out=XK[:], in_=xT_ext[:])
            FLD = pp.tile([128, 32], F32)  # fold matrix: FLD[k, b] = (k%32 == b)
            make_identity(nc, FLD[0:32, :])
            for r in range(1, 4):
                nc.scalar.copy(FLD[32 * r : 32 * (r + 1), :], FLD[0:32, :])
            with (
                tc.tile_pool(name="wpool", bufs=wbufs) as wp,
                tc.tile_pool(name="psum", bufs=1, space="PSUM") as psp,
            ):
                for g in range(NG):
                    # lhsT packs the group's 4 s-values block-column-wise:
                    # column 32*j+b holds x[b, 4g+j, :]. Each s's matmul then
                    # computes a full [128, N] product of which rows
                    # 32j..32j+32 are that s's u_hat (f32r requires PSUM
                    # base partition 0, so col-tiling is unavailable).
                    for j in range(4):  # s within group
                        ps = psp.tile([128, CO], F32, tag=f"ps{j}")
                        wt = wp.tile([128, KI, CO], F16, tag="wt")
                        eng = (
                            nc.sync
                            if dma_spread == 0
                            else [nc.sync, nc.gpsimd, nc.vector, nc.scalar][
                                j % dma_spread
                            ]
                        )
                        eng.dma_start(
                            out=wt[:],
                            in_=wT_ext[4 * g + j].rearrange(
                                "(k p) n -> p k n", p=128
                            ),
                        )
                        for n in range(2):  # CO split into 2x512
                            for ki in range(KI):
                                nc.tensor.matmul(
                                    ps[:, 512 * n : 512 * (n + 1)],
                                    XK[:, ki, 4 * g : 4 * g + 4, :],
                                    wt[:, ki, 512 * n : 512 * (n + 1)],
                                    start=(ki == 0),
                                    stop=(ki == KI - 1),
                                )
                        nc.scalar.copy(
                            U[32 * j : 32 * (j + 1), g, :, :],
                            ps[32 * j : 32 * (j + 1), :],
                        )
                    if g == 0:
                        nc.vector.tensor_copy(ACC[:], U[:, 0, :, :])
                    else:
                        nc.vector.tensor_add(ACC[:], ACC[:], U[:, g, :, :])

            with tc.tile_pool(name="psum2", bufs=1, space="PSUM") as psp2:
                # ---------------- phase 2: routing ----------------
                # All routing is split into two capsule halves (c 0..7 / 8..15) so
                # each half's AllReduce overlaps the other half's DVE work. The
                # softmax couples the halves (normalizes over all 16 capsules).
                CH = C // 2          # capsules per half
                FH = CO // 2         # flat (c,o) elements per half
                T = pp.tile([128, NG // 2, C, DOUT], F32)  # cumsum scratch (8 slots)
                BL = pp.tile([128, NG, C], F32)        # b_ij logits
                BI = pp.tile([128, NG, C], F32)        # agreement increment
                CI = pp.tile([128, NG, C], F32)        # c_ij
                Mx = pp.tile([128, NG], F32)
                Zs = pp.tile([128, NG], F32)
                Rz = pp.tile([128, NG], F32)
                AG = pp.tile([128, CO], F32)
                VR = pp.tile([128, CO], F32)           # v_j replicated over j
                XH = [pp.tile([32, FH], F32, tag=f"xh{h}", name=f"xh{h}") for h in range(2)]
                SPH = [pp.tile([32, FH], F32, tag=f"sph{h}", name=f"sph{h}") for h in range(2)]
                SJH = [pp.tile([32, FH], F32, tag=f"sjh{h}", name=f"sjh{h}") for h in range(2)]
                N2H = [pp.tile([32, CH], F32, tag=f"n2h{h}", name=f"n2h{h}") for h in range(2)]
                NyH = [pp.tile([32, CH], F32, tag=f"nyh{h}", name=f"nyh{h}") for h in range(2)]
                RyH = [pp.tile([32, CH], F32, tag=f"ryh{h}", name=f"ryh{h}") for h in range(2)]
                NTH = [pp.tile([32, CH], F32, tag=f"nth{h}", name=f"nth{h}") for h in range(2)]
                Y2H = [pp.tile([32, CH], F32, tag=f"y2h{h}", name=f"y2h{h}") for h in range(2)]
                DnH = [pp.tile([32, CH], F32, tag=f"dnh{h}", name=f"dnh{h}") for h in range(2)]
                RdH = [pp.tile([32, CH], F32, tag=f"rdh{h}", name=f"rdh{h}") for h in range(2)]
                FsH = [pp.tile([32, CH], F32, tag=f"fsh{h}", name=f"fsh{h}") for h in range(2)]

                def fsl(h):
                    return slice(FH * h, FH * (h + 1))

                def csl(h):
                    return slice(CH * h, CH * (h + 1))

                def partial_from_h(src_ap, h):
                    """[128, FH] partition-reduce over j on the (idle) PE:
                    SP[b, n] = sum_k FLD[k, b] * src[k, n]. Returns the PSUM
                    tile; the AllReduce DMA reads it directly."""
                    ps_sp = psp2.tile([32, FH], F32, tag=f"spps{h}", name=f"spps{h}")
                    nc.tensor.matmul(ps_sp[:], FLD[:], src_ap, start=True, stop=True)
                    nc.scalar.copy(SPH[h][:], ps_sp[:])  # DMA cannot read PSUM
                    return SPH[h]

                def allreduce_h(k, h, src_ps):
                    idx = 2 * k + h
                    nc.sync.dma_start(out=cc_in[idx][:], in_=src_ps[:])
                    if sim_local:
                        # TimelineSim can't model collectives; stand-in DMA.
                        nc.sync.dma_start(out=cc_out[idx][:], in_=cc_in[idx][:])
                    else:
                        nc.gpsimd.collective_compute(
                            "AllReduce", ALU.add,
                            replica_groups=groups,
                            ins=[cc_in[idx][:]],
                            outs=[cc_out[idx][:]],
                        )
                    nc.sync.dma_start(out=SJH[h][:], in_=cc_out[idx][:])

                def squash_h(h, last):
                    """SJH[h] [32,(c8,o)] -> v_j half; into VR[0:32, half],
                    replicated across j. sqrt via exp(0.5*ln) (one ACT table set
                    with softmax's exp) + one Newton step."""
                    SJh, X = SJH[h], XH[h]
                    n2, ny, ry, nt, y2 = N2H[h], NyH[h], RyH[h], NTH[h], Y2H[h]
                    dn, rd, fsv = DnH[h], RdH[h], FsH[h]
                    sj_c = SJh[:].rearrange("p (c o) -> p c o", c=CH)
                    nc.vector.tensor_mul(X[:], SJh[:], SJh[:])
                    nc.vector.tensor_reduce(
                        n2[:], X[:].rearrange("p (c o) -> p c o", c=CH),
                        axis=AX.X, op=ALU.add,
                    )
                    nc.scalar.activation(ny[:], n2[:], ACTF.Ln)
                    nc.scalar.activation(ny[:], ny[:], ACTF.Exp, scale=0.5)
                    # Newton: y = 0.5*(y0 + n2/y0)
                    nc.vector.reciprocal(ry[:], ny[:])
                    nc.vector.tensor_mul(nt[:], n2[:], ry[:])
                    nc.vector.tensor_add(y2[:], ny[:], nt[:])
                    nc.vector.tensor_scalar_mul(y2[:], y2[:], 0.5)
                    # f = y / (1 + n2);  v = s_j * f
                    nc.vector.tensor_scalar_add(dn[:], n2[:], 1.0)
                    nc.vector.reciprocal(rd[:], dn[:])
                    nc.vector.tensor_mul(fsv[:], y2[:], rd[:])
                    vr_c = VR[0:32, fsl(h)].rearrange("p (c o) -> p c o", c=CH)
                    nc.vector.tensor_mul(vr_c, sj_c, fsv[:].broadcast_to([32, CH, DOUT]))
                    if last:
                        nc.sync.dma_start(out=out_ext[:, fsl(h)], in_=VR[0:32, fsl(h)])
                    else:
                        for r in range(1, 4):
                            nc.scalar.copy(
                                VR[32 * r : 32 * (r + 1), fsl(h)], VR[0:32, fsl(h)]
                            )

                def agreement_h(h, first):
                    """BL/BI[:, :, c-half] = sum_o u_hat*v via fused mul-cumsum.

                    Per g: one MUL_CUMSUM over the flat (c-half, o) stream;
                    per-capsule sums recovered by differencing the cumsum at o=63
                    (fp32 cancellation error ~2^-24*|running sum|, negligible)."""
                    dst = BL if first else BI
                    cs = csl(h)
                    for bb in range(2):  # g in two batches of 8 (T has 8 slots)
                        for gg in range(8):
                            g = 8 * bb + gg
                            nc.vector._custom_dve(
                                MUL_CUMSUM,
                                out=T[:, gg, 0:CH, :].rearrange("p c o -> p (c o)"),
                                in0=U[:, g, cs, :].rearrange("p c o -> p (c o)"),
                                in1=VR[:, fsl(h)],
                            )
                        cum63 = T[:, :, 0:CH, DOUT - 1]  # [p, 8, CH]
                        gs = slice(8 * bb, 8 * (bb + 1))
                        nc.vector.tensor_copy(
                            dst[:, gs, CH * h : CH * h + 1], cum63[:, :, 0:1]
                        )
                        nc.vector.tensor_sub(
                            dst[:, gs, CH * h + 1 : CH * (h + 1)],
                            cum63[:, :, 1:],
                            cum63[:, :, 0 : CH - 1],
                        )

                def softmax():
                    nc.vector.tensor_reduce(Mx[:], BL[:], axis=AX.X, op=ALU.max)
                    nc.vector.tensor_sub(CI[:], BL[:], Mx[:].broadcast_to([128, NG, C]))
                    nc.scalar.activation(CI[:], CI[:], ACTF.Exp)
                    nc.vector.tensor_reduce(Zs[:], CI[:], axis=AX.X, op=ALU.add)
                    nc.vector.reciprocal(Rz[:], Zs[:])
                    nc.vector.tensor_mul(CI[:], CI[:], Rz[:].broadcast_to([128, NG, C]))

                def weighted_sum_h(h):
                    """AG[p, c-half, :] = sum_g u_hat*c_ij via fused mul-cumsum
                    (per c: (o,g) stream, g innermost; diff at g=15), then
                    partition-reduce over j into SPH[h]."""
                    agv = AG[:].rearrange("p (c o) -> p c o", c=C)
                    for cc in range(CH):
                        c = CH * h + cc
                        nc.vector._custom_dve(
                            MUL_CUMSUM,
                            out=T[:, cc, :, :]
                            .rearrange("p c o -> p (c o)")
                            .rearrange("p (o g) -> p o g", o=DOUT),
                            in0=U[:, :, c, :].rearrange("p g o -> p o g"),
                            in1=CI[:, :, c]
                            .broadcast_to([128, NG, DOUT])
                            .rearrange("p g o -> p o g"),
                        )
                    cum15 = (
                        T[:]
                        .rearrange("p h c o -> p h (c o)")
                        .rearrange("p h (o g) -> p h o g", o=DOUT)[:, :, :, NG - 1]
                    )  # [p, 8, DOUT]
                    cs = csl(h)
                    nc.vector.tensor_copy(agv[:, cs, 0:1], cum15[:, :, 0:1])
                    nc.vector.tensor_sub(
                        agv[:, cs, 1:], cum15[:, :, 1:], cum15[:, :, 0 : DOUT - 1]
                    )
                    return partial_from_h(AG[:, fsl(h)], h)

                if skip_routing:
                    nc.sync.dma_start(out=out_ext[:], in_=ACC[0:32, :])
                    nc.compile()
                    return nc

                # ---- iter 0: c_ij uniform = 1/C ----
                for h in range(2):
                    ps = partial_from_h(ACC[:, fsl(h)], h)
                    allreduce_h(0, h, ps)
                for h in range(2):
                    # fold the uniform 1/C weight in after the (linear) AR
                    nc.vector.tensor_scalar_mul(SJH[h][:], SJH[h][:], 1.0 / C)
                    squash_h(h, last=False)
                    agreement_h(h, first=True)

                # ---- iter 1 ----
                softmax()
                for h in range(2):
                    ps = weighted_sum_h(h)
                    allreduce_h(1, h, ps)
                for h in range(2):
                    squash_h(h, last=False)
                    agreement_h(h, first=False)
                nc.vector.tensor_add(BL[:], BL[:], BI[:])

                # ---- iter 2 ----
                # The final cross-core reduce + squash are part of the host
                # unshard: each core emits its local sum_s c_ij*u_hat partial
                # (the last AllReduce would sit fully exposed at the kernel
                # tail with no DVE work left to hide it).
                softmax()
                for h in range(2):
                    ps = weighted_sum_h(h)
                    nc.sync.dma_start(out=out_ext[:, fsl(h)], in_=ps[:])

    nc.compile()
    return nc


def _get_nc():
    if "nc" not in _CACHE:
        _CACHE["nc"] = _build()
    return _CACHE["nc"]


def _get_runner():
    """Cached shard_map executable over the 8 cores (mirrors
    bass2jax.run_bass_via_pjrt, but reusable across calls and without the
    per-core concat — the s-outer host layout makes the global concatenated
    input exactly xT/wT)."""
    if "runner" in _CACHE:
        return _CACHE["runner"]
    import jax
    from jax.sharding import Mesh, PartitionSpec
    from jax.experimental.shard_map import shard_map
    from concourse import bass2jax as b2j

    nc = _get_nc()
    b2j.install_neuronx_cc_hook()
    partition_name = nc.partition_id_tensor.name if nc.partition_id_tensor else None
    in_names, out_names, out_avals = [], [], []
    for alloc in nc.m.functions[0].allocations:
        if not isinstance(alloc, mybir.MemoryLocationSet):
            continue
        name = alloc.memorylocations[0].name
        if alloc.kind == "ExternalInput":
            if name != partition_name:
                in_names.append(name)
        elif alloc.kind == "ExternalOutput":
            out_names.append(name)
            out_avals.append(
                jax.core.ShapedArray(tuple(alloc.tensor_shape), mybir.dt.np(alloc.dtype))
            )
    n_params = len(in_names)
    all_in_names = list(in_names) + list(out_names)
    if partition_name is not None:
        all_in_names.append(partition_name)

    def _body(*args):
        operands = list(args)
        if partition_name is not None:
            operands.append(b2j.partition_id_tensor())
        outs = b2j._bass_exec_p.bind(
            *operands,
            out_avals=tuple(out_avals),
            in_names=tuple(all_in_names),
            out_names=tuple(out_names),
            lowering_input_output_aliases=(),
            sim_require_finite=True,
            sim_require_nnan=True,
            nc=nc,
        )
        return tuple(outs)

    devices = jax.devices()[:NCORES]
    mesh = Mesh(np.asarray(devices), ("core",))
    n_outs = len(out_names)
    sharded = jax.jit(
        shard_map(
            _body,
            mesh=mesh,
            in_specs=(PartitionSpec("core"),) * (n_params + n_outs),
            out_specs=(PartitionSpec("core"),) * n_outs,
            check_rep=False,
        ),
        donate_argnums=tuple(range(n_params, n_params + n_outs)),
        keep_unused=True,
    )
    _CACHE["runner"] = (sharded, in_names, out_names, out_avals)
    return _CACHE["runner"]


def kernel(x: np.ndarray, W: np.ndarray) -> np.ndarray:
    assert x.shape == (B, S, DIN) and W.shape == (C, S, DIN, DOUT)
    xf = x.astype(np.float32)
    xk = np.empty((NCORES * 128, KI, S_LOC, B), np.float16)
    for c in range(NCORES):
        sl = xf[:, c * S_LOC : (c + 1) * S_LOC, :]  # [B, S_LOC, DIN]
        for ki in range(KI):
            xk[c * 128 : (c + 1) * 128, ki] = sl[
                :, :, ki * 128 : (ki + 1) * 128
            ].transpose(2, 1, 0)
    wT = np.ascontiguousarray(
        np.transpose(W.astype(np.float32), (1, 2, 0, 3)).reshape(S, DIN, CO)
    ).astype(np.float16)
    sharded, in_names, out_names, out_avals = _get_runner()
    ins = {"xT": xk, "wT": wT}
    concat_in = [ins[name] for name in in_names]
    concat_zeros = [
        np.zeros((NCORES * a.shape[0], *a.shape[1:]), a.dtype) for a in out_avals
    ]
    out_arrs = sharded(*concat_in, *concat_zeros)
    parts = np.asarray(out_arrs[out_names.index("out")]).reshape(NCORES, B, C, DOUT)
    s_j = parts.astype(np.float64).sum(axis=0)
    n2 = np.sum(s_j * s_j, axis=-1, keepdims=True)
    n = np.sqrt(n2)
    v = n / (1.0 + n2) * s_j
    return np.ascontiguousarray(v.astype(np.float32))



# revision 16
# speedup vs baseline: 1.0989x; 1.0989x over previous
"""CapsuleLayer (dynamic routing, 3 iters) on 8 TRN2 NeuronCores.

Strategy: shard the num_routes axis S=512 into 64 s-values per core.

Phase 1 (DMA-bound, ~100us): u_hat[b,s_loc,(o,c)] = x[b,s,:] @ W[s][:, (o,c)]
  streamed from HBM (W is 32 MiB/core in f16 — f16 is required: the routing
  softmax acts as a near-argmax over logits of magnitude ~1e2, so fp8 u_hat
  error (~2%) flips routing decisions and blows the 2e-2 gate).
  lhsT x-operands are zero-padded block-diagonal (XKZ[j] has x for s=4g+j in
  columns 32j..32j+32, zeros elsewhere) so the 4 s-values of a group
  accumulate into ONE [128, CO] PSUM tile -> one full-width ACT copy per
  group into U (f16, layout [p, g, o, c] — c innermost enables the DVE
  16-bit 2x mode for the weighted-sum multiply in phase 2).

Phase 2 (routing): partition p = 32*j + b.
  - agreement: fused MUL_CUMSUM over the (c,o) stream per g (f16 in, f32
    cumsum out), per-capsule sums recovered by differencing at o=63.
  - softmax over capsules: local (c on the free axis).
  - weighted sum: P = c_ij * u_hat as an all-f16 packed TensorTensor (DVE 2x
    mode), then the sum over (j, g) runs on the idle PE: 0/1 fold matrix
    FLD as lhsT, PSUM-accumulating over g. Only s_j = [B,CO] crosses cores:
    one AllReduce (two capsule halves) per routing iteration, 4 total; the
    final iteration emits per-core partials reduced on the host.
"""
import numpy as np

import concourse.bass as bass
import concourse.mybir as mybir
import concourse.tile as tile
from concourse import bacc
from concourse.bass_utils import run_bass_kernel_spmd
from concourse.masks import make_identity

B, S, C, DIN, DOUT = 32, 512, 16, 256, 64
NCORES = 8
S_LOC = S // NCORES          # 64
NG = S_LOC // 4              # 16 groups of 4 s-values
CO = C * DOUT                # 1024
CH = C // 2                  # capsules per half
KI = DIN // 128              # 2 contraction chunks
F32 = mybir.dt.float32
F32R = mybir.dt.float32r
F16 = mybir.dt.float16
AX = mybir.AxisListType
ALU = mybir.AluOpType
ACTF = mybir.ActivationFunctionType

_CACHE = {}


def _register_mul_cumsum():
    """out[p, :] = running cumsum of in0*in1 along the free stream.

    Registered at runtime (dve_ops.py is read-only here); same mechanism as
    the production ops — the per-NEFF DVE table is generated from OPS by
    name at compile time."""
    from concourse import dve_ops
    from concourse.dve_spec import Spec, Src0, Src1, AluOp, scan, lower as dve_lower
    from concourse.dve_uop import DveOpSpec

    name = "MUL_CUMSUM_ANT"
    for op in dve_ops.OPS:
        if op.name == name:
            return op

    def _ref(in0, in1, s0, s1, imm2):
        prod = (np.asarray(in0, np.float32) * np.asarray(in1, np.float32)).astype(
            np.float32
        )
        flat = prod.reshape(prod.shape[0], -1)
        return np.cumsum(flat, axis=1, dtype=np.float32).reshape(prod.shape)

    spec = Spec(body=scan(AluOp.ADD, Src0 * Src1), reference=_ref)
    row = dve_ops._CUSTOM_DVE_ROW_BASE + len(dve_ops.OPS)
    assert row < 0x20
    dve_ops._SUB_OPCODE_FOR_NAME[name] = row
    shas = {}
    for ver in ("v3", "v4"):
        uops = dve_lower(spec, ver=ver)
        shas[ver] = DveOpSpec(name=name, opcode=row, uops=uops, rd1_en=True).sha(ver)
    op = dve_ops.DveOp(name, spec, subdim=False, uops_sha=shas)
    dve_ops.OPS.append(op)
    dve_ops.CUSTOM_DVE_SPECS[name] = spec
    return op


MUL_CUMSUM = _register_mul_cumsum()


def _patch_act_tables():
    """Steer the act-table chooser to the ln+exp+copy set so squash (Ln,
    Exp) and softmax (Exp) share one table: without this the first-match rule
    alternates exp_and_others/natural_log, costing a 1.3us table load per
    switch. The combined set is a real act_info.json entry, so the emitted
    set id loads a table that genuinely contains every function used."""
    import concourse.bacc as _bacc_mod
    if getattr(_bacc_mod, "_ant_act_patched", False):
        return
    _orig = _bacc_mod.get_activation_tables

    def _patched(arch):
        out = {}
        for name, funcs in _orig(arch).items():
            if name != "natural_log_exp_and_others":
                funcs = funcs - {ACTF.Exp, ACTF.Ln}
            out[name] = funcs
        return out

    _bacc_mod.get_activation_tables = _patched
    _bacc_mod._ant_act_patched = True


_patch_act_tables()


def _build(sim_local=False, skip_routing=False, wbufs=3, dma_spread=0):
    nc = bacc.Bacc("TRN2", target_bir_lowering=False, debug=False, num_devices=NCORES)
    # Host pre-transposed inputs (per-core shards):
    #   xT: [128, KI, S_LOC, B] (partition = din within ki-chunk)
    #   wT: [S_LOC, DIN, DOUT*C]  — note (o, c) column order
    xT_ext = nc.declare_dram_parameter("xT", [128, KI, S_LOC, B], F16, isOutput=False)
    wT_ext = nc.declare_dram_parameter("wT", [S_LOC, DIN, CO], F16, isOutput=False)
    out_ext = nc.declare_dram_parameter("out", [B, CO], F32, isOutput=True)

    cc_in = [nc.dram_tensor(f"cc_in{k}", [B, CO // 2], F32) for k in range(4)]
    cc_out = [
        nc.dram_tensor(f"cc_out{k}", [B, CO // 2], F32, addr_space="Shared")
        for k in range(4)
    ]
    groups = [list(range(NCORES))]

    with tile.TileContext(nc) as tc:
        with (
            tc.tile_pool(name="persist", bufs=1) as pp,
            tc.tile_pool(name="psumP", bufs=1, space="PSUM") as pspP,
        ):
            # ---------------- phase 1: u_hat ----------------
            U = pp.tile([128, NG, DOUT, C], F16)      # u_hat, 32 KiB/part
            # Zero-padded block-diagonal lhsT: XKZ[:, j, ki, g, 32j:32j+32]
            # holds x[b, 4g+j, (ki,p)]; other columns zero. The 8 (j, ki)
            # matmuls of a group then accumulate the group's 4 u_hat rowsets
            # into one PSUM tile.
            XKZ = pp.tile([128, 4, KI, NG, 128], F16)
            nc.vector.memset(XKZ[:, 0:2], 0)
            nc.gpsimd.memset(XKZ[:, 2:4], 0)
            for j in range(4):
                nc.sync.dma_start(
                    out=XKZ[:, j, :, :, 32 * j : 32 * (j + 1)],
                    in_=xT_ext[:, :, j::4, :],
                )
            FLD = pp.tile([128, 32], F16)  # fold matrix: FLD[k, b] = (k%32 == b)
            make_identity(nc, FLD[0:32, :])
            for r in range(1, 4):
                nc.scalar.copy(FLD[32 * r : 32 * (r + 1), :], FLD[0:32, :])
            # Warm the (single) activation table under the DMA shadow so the
            # 1.3us LoadActFuncSet doesn't land on the critical path at AR0.
            WRM = pp.tile([1, 1], F32)
            nc.vector.memset(WRM[:], 1.0)
            nc.scalar.activation(WRM[:], WRM[:], ACTF.Ln)
            ps0 = pspP.tile([32, CO], F32, tag="ps0", name="ps0")
            with (
                tc.tile_pool(name="wpool", bufs=wbufs) as wp,
                tc.tile_pool(name="psum", bufs=2, space="PSUM") as psp,
            ):
                for g in range(NG):
                    wtg = wp.tile([128, 4, KI, CO], F16, tag="wt")
                    nc.sync.dma_start(
                        out=wtg[:],
                        in_=wT_ext[4 * g : 4 * (g + 1)].rearrange(
                            "s (k p) n -> p s k n", p=128
                        ),
                    )
                    ps = psp.tile([128, CO], F32, tag="ps")
                    for nh in range(2):
                        for j in range(4):
                            for ki in range(KI):
                                nc.tensor.matmul(
                                    ps[:, 512 * nh : 512 * (nh + 1)],
                                    XKZ[:, j, ki, g, :],
                                    wtg[:, j, ki, 512 * nh : 512 * (nh + 1)],
                                    start=(j == 0 and ki == 0),
                                    stop=(j == 3 and ki == KI - 1),
                                    skip_group_check=True,
                                )
                    nc.scalar.copy(
                        U[:, g].rearrange("p o c -> p (o c)"), ps[:]
                    )
                    # iter-0 partial sum over local s: fold (j, g) on the PE
                    # as PSUM-accumulated FLD matmuls over U. Emitted as three
                    # closed batches over groups whose U copies are already
                    # done, so the scheduler can run the first two under the
                    # DMA shadow instead of piling 32 matmuls on the tail.
                    batch = {9: range(0, 8), 14: range(8, 13), 15: range(13, 16)}
                    for gf in batch.get(g, ()):
                        for nh in range(2):
                            nc.tensor.matmul(
                                ps0[:, 512 * nh : 512 * (nh + 1)],
                                FLD[:],
                                U[:, gf].rearrange("p o c -> p (o c)")[
                                    :, 512 * nh : 512 * (nh + 1)
                                ],
                                start=(gf == 0),
                                stop=(gf == NG - 1),
                                skip_group_check=True,
                            )

            with (
                tc.tile_pool(name="ppool", bufs=6) as ppl,
                tc.tile_pool(name="psum2", bufs=1, space="PSUM") as psp2,
            ):
                # ---------------- phase 2: routing ----------------
                T = pp.tile([128, NG, C, DOUT], F32)   # cumsum scratch (full U)
                BL = pp.tile([128, NG, C], F32)        # b_ij logits
                BI = pp.tile([128, NG, C], F32)        # agreement increment
                SM = pp.tile([128, NG, C], F32)        # softmax scratch
                CI = pp.tile([128, NG, C], F16)        # c_ij (f16 for 2x P-mul)
                Mx = pp.tile([128, NG], F32)
                Zs = pp.tile([128, NG], F32)
                Rz = pp.tile([128, NG], F32)
                VR = pp.tile([128, C, DOUT], F16)      # v_j (c-major)
                SPH = pp.tile([32, C, DOUT], F32)      # AR send staging (c-major)
                SJH = pp.tile([32, C, DOUT], F32)      # AR result (s_j, c-major)
                # squash scratch (partitions 0:32 = batch)
                N2 = pp.tile([32, C], F32)
                Ny = pp.tile([32, C], F32)
                Ry = pp.tile([32, C], F32)
                Nt = pp.tile([32, C], F32)
                Y2 = pp.tile([32, C], F32)
                Dn = pp.tile([32, C], F32)
                Rd = pp.tile([32, C], F32)
                Fs = pp.tile([32, C], F32)
                XS = pp.tile([32, C, DOUT], F32)

                def allreduce_h(k, h):
                    """AllReduce one capsule-half of SPH into SJH; halves run
                    on independent DMA queues (sync, scalar) so the two
                    send/reduce/land chains overlap, and squash of half 0
                    hides under half 1's chain."""
                    engs = [nc.sync, nc.scalar]
                    idx = 2 * k + h
                    cs = slice(CH * h, CH * (h + 1))
                    engs[h].dma_start(
                        out=cc_in[idx][:],
                        in_=SPH[:, cs].rearrange("p c o -> p (c o)"),
                    )
                    if sim_local:
                        # TimelineSim can't model collectives; stand-in DMA.
                        engs[h].dma_start(out=cc_out[idx][:], in_=cc_in[idx][:])
                    else:
                        nc.gpsimd.collective_compute(
                            "AllReduce", ALU.add,
                            replica_groups=groups,
                            ins=[cc_in[idx][:]],
                            outs=[cc_out[idx][:]],
                        )
                    engs[h].dma_start(
                        out=SJH[:, cs].rearrange("p c o -> p (c o)"),
                        in_=cc_out[idx][:],
                    )

                def squash_h(h):
                    """One capsule-half of SJH -> v_j into VR[0:32, half],
                    then replicate that half across the j partition blocks on
                    ACT (so DVE can squash the other half concurrently).
                    sqrt via exp(0.5*ln) + one Newton step."""
                    cs = slice(CH * h, CH * (h + 1))
                    sj = SJH[:, cs]
                    n2, ny, ry, nt = N2[:, cs], Ny[:, cs], Ry[:, cs], Nt[:, cs]
                    y2, dn, rd, fs = Y2[:, cs], Dn[:, cs], Rd[:, cs], Fs[:, cs]
                    nc.vector.tensor_mul(XS[:, cs], sj, sj)
                    nc.vector.tensor_reduce(n2, XS[:, cs], axis=AX.X, op=ALU.add)
                    nc.scalar.activation(ny, n2, ACTF.Ln)
                    nc.scalar.activation(ny, ny, ACTF.Exp, scale=0.5)
                    # Newton: y = 0.5*(y0 + n2/y0)
                    nc.vector.reciprocal(ry, ny)
                    nc.vector.tensor_mul(nt, n2, ry)
                    nc.vector.tensor_add(y2, ny, nt)
                    nc.vector.tensor_scalar_mul(y2, y2, 0.5)
                    # f = y / (1 + n2);  v = s_j * f
                    nc.vector.tensor_scalar_add(dn, n2, 1.0)
                    nc.vector.reciprocal(rd, dn)
                    nc.vector.tensor_mul(fs, y2, rd)
                    nc.vector.tensor_mul(
                        VR[0:32, cs], sj, fs.broadcast_to([32, CH, DOUT])
                    )

                def replicate_v():
                    # replicate v across the 3 other j partition blocks,
                    # full-width, split across ACT and DVE
                    nc.scalar.copy(VR[32:64], VR[0:32])
                    nc.vector.tensor_copy(VR[64:96], VR[0:32])
                    nc.vector.tensor_copy(VR[96:128], VR[0:32])

                def agreement(first):
                    """dst[:, g, c] = sum_o u_hat*v via ONE fused mul-cumsum
                    over the whole (g, c, o) stream; per-(g,c) sums recovered
                    by differencing the f32 cumsum at the o=63 positions.
                    (Adjacent cum values share their long prefix, so the
                    differencing error is only the within-segment rounding,
                    ~sqrt(64)*ulp — negligible in f32.)"""
                    dst = BL if first else BI
                    for g in range(NG):
                        nc.vector._custom_dve(
                            MUL_CUMSUM,
                            out=T[:, g].rearrange("p c o -> p (c o)"),
                            in0=U[:, g].rearrange("p o c -> p c o"),
                            in1=VR[:],
                        )
                    cum63 = T[:, :, :, DOUT - 1]  # [p, g, c]
                    nc.vector.tensor_copy(dst[:, :, 0:1], cum63[:, :, 0:1])
                    nc.vector.tensor_sub(
                        dst[:, :, 1:], cum63[:, :, 1:], cum63[:, :, 0 : C - 1]
                    )

                def softmax(from_bl_plus_bi):
                    src = BL
                    if from_bl_plus_bi:
                        nc.vector.tensor_add(BL[:], BL[:], BI[:])
                    nc.vector.tensor_reduce(Mx[:], src[:], axis=AX.X, op=ALU.max)
                    nc.vector.tensor_sub(SM[:], src[:], Mx[:].broadcast_to([128, NG, C]))
                    nc.scalar.activation(CI[:], SM[:], ACTF.Exp)
                    nc.vector.tensor_reduce(Zs[:], CI[:], axis=AX.X, op=ALU.add)
                    nc.vector.reciprocal(Rz[:], Zs[:])
                    nc.vector.tensor_mul(CI[:], CI[:], Rz[:].broadcast_to([128, NG, C]))

                def weighted_sum(tag):
                    """psum[b, (o,c)] = sum_{j,g} c_ij*u_hat: per-g f16 2x-mode
                    multiply on DVE, then the (j, g)-fold runs on the idle PE
                    (FLD as lhsT, PSUM accumulation over g)."""
                    psw = psp2.tile([32, CO], F32, tag=tag, name=tag)
                    for g in range(NG):
                        Pt = ppl.tile([128, DOUT, C], F16, tag="P")
                        nc.vector.tensor_mul(
                            Pt[:],
                            U[:, g],
                            CI[:, g, :]
                            .broadcast_to([128, C, DOUT])
                            .rearrange("p c o -> p o c"),
                        )
                        for nh in range(2):
                            nc.tensor.matmul(
                                psw[:, 512 * nh : 512 * (nh + 1)],
                                FLD[:],
                                Pt[:].rearrange("p o c -> p (o c)")[
                                    :, 512 * nh : 512 * (nh + 1)
                                ],
                                start=(g == 0),
                                stop=(g == NG - 1),
                            )
                    return psw

                if skip_routing:
                    nc.scalar.copy(SPH[:], ps0[:])
                    nc.sync.dma_start(out=out_ext[:], in_=SPH[:])
                    nc.compile()
                    return nc

                def stage_and_reduce(k, psrc, scale):
                    """PSUM -> SPH (c-major transpose copy) per half, AR both
                    halves on separate queues, squash each half as it lands."""
                    pv = psrc[:].rearrange("b (o c) -> b o c", o=DOUT)
                    for h in range(2):
                        cs = slice(CH * h, CH * (h + 1))
                        if scale is None:
                            nc.scalar.copy(
                                SPH[:, cs], pv[:, :, cs].rearrange("b o c -> b c o")
                            )
                        else:
                            nc.scalar.mul(
                                SPH[:, cs],
                                pv[:, :, cs].rearrange("b o c -> b c o"),
                                scale,
                            )
                        allreduce_h(k, h)
                    for h in range(2):
                        squash_h(h)
                    replicate_v()

                # ---- iter 0: c_ij uniform = 1/C ----
                stage_and_reduce(0, ps0, 1.0 / C)
                agreement(first=True)

                # ---- iter 1 ----
                softmax(from_bl_plus_bi=False)
                psw = weighted_sum("psw1")
                stage_and_reduce(1, psw, None)
                agreement(first=False)

                # ---- iter 2 ----
                # The final cross-core reduce + squash are part of the host
                # unshard: each core emits its local sum_s c_ij*u_hat partial.
                softmax(from_bl_plus_bi=True)
                psw2 = weighted_sum("psw2")
                OUTS = pp.tile([32, CO], F32)
                engs = [nc.sync, nc.scalar]
                for h in range(2):
                    cols = slice(512 * h, 512 * (h + 1))
                    nc.scalar.copy(OUTS[:, cols], psw2[:, cols])
                    engs[h].dma_start(out=out_ext[:, cols], in_=OUTS[:, cols])

    nc.compile()
    return nc


def _get_nc():
    if "nc" not in _CACHE:
        _CACHE["nc"] = _build()
    return _CACHE["nc"]


def _get_runner():
    """Cached shard_map executable over the 8 cores (mirrors
    bass2jax.run_bass_via_pjrt, but reusable across calls and without the
    per-core concat — the s-outer host layout makes the global concatenated
    input exactly xT/wT)."""
    if "runner" in _CACHE:
        return _CACHE["runner"]
    import jax
    from jax.sharding import Mesh, PartitionSpec
    from jax.experimental.shard_map import shard_map
    from concourse import bass2jax as b2j

    nc = _get_nc()
    b2j.install_neuronx_cc_hook()
    partition_name = nc.partition_id_tensor.name if nc.partition_id_tensor else None
    in_names, out_names, out_avals = [], [], []
    for alloc in nc.m.functions[0].allocations:
        if not isinstance(alloc, mybir.MemoryLocationSet):
            continue
        name = alloc.memorylocations[0].name
        if alloc.kind == "ExternalInput":
            if name != partition_name:
                in_names.append(name)
        elif alloc.kind == "ExternalOutput":
            out_names.append(name)
            out_avals.append(
                jax.core.ShapedArray(tuple(alloc.tensor_shape), mybir.dt.np(alloc.dtype))
            )
    n_params = len(in_names)
    all_in_names = list(in_names) + list(out_names)
    if partition_name is not None:
        all_in_names.append(partition_name)

    def _body(*args):
        operands = list(args)
        if partition_name is not None:
            operands.append(b2j.partition_id_tensor())
        outs = b2j._bass_exec_p.bind(
            *operands,
            out_avals=tuple(out_avals),
            in_names=tuple(all_in_names),
            out_names=tuple(out_names),
            lowering_input_output_aliases=(),
            sim_require_finite=True,
            sim_require_nnan=True,
            nc=nc,
        )
        return tuple(outs)

    devices = jax.devices()[:NCORES]
    mesh = Mesh(np.asarray(devices), ("core",))
    n_outs = len(out_names)
    sharded = jax.jit(
        shard_map(
            _body,
            mesh=mesh,
            in_specs=(PartitionSpec("core"),) * (n_params + n_outs),
            out_specs=(PartitionSpec("core"),) * n_outs,
            check_rep=False,
        ),
        donate_argnums=tuple(range(n_params, n_params + n_outs)),
        keep_unused=True,
    )
    _CACHE["runner"] = (sharded, in_names, out_names, out_avals)
    return _CACHE["runner"]


def kernel(x: np.ndarray, W: np.ndarray) -> np.ndarray:
    assert x.shape == (B, S, DIN) and W.shape == (C, S, DIN, DOUT)
    xf = x.astype(np.float32)
    xk = np.empty((NCORES * 128, KI, S_LOC, B), np.float16)
    for c in range(NCORES):
        sl = xf[:, c * S_LOC : (c + 1) * S_LOC, :]  # [B, S_LOC, DIN]
        for ki in range(KI):
            xk[c * 128 : (c + 1) * 128, ki] = sl[
                :, :, ki * 128 : (ki + 1) * 128
            ].transpose(2, 1, 0)
    # W columns in (o, c) order: wT[s, i, (o, c)]
    wT = np.ascontiguousarray(
        np.transpose(W.astype(np.float32), (1, 2, 3, 0)).reshape(S, DIN, CO)
    ).astype(np.float16)
    sharded, in_names, out_names, out_avals = _get_runner()
    ins = {"xT": xk, "wT": wT}
    concat_in = [ins[name] for name in in_names]
    concat_zeros = [
        np.zeros((NCORES * a.shape[0], *a.shape[1:]), a.dtype) for a in out_avals
    ]
    out_arrs = sharded(*concat_in, *concat_zeros)
    parts = np.asarray(out_arrs[out_names.index("out")]).reshape(
        NCORES, B, DOUT, C
    )
    s_j = parts.astype(np.float64).sum(axis=0).transpose(0, 2, 1)  # [B, C, DOUT]
    n2 = np.sum(s_j * s_j, axis=-1, keepdims=True)
    n = np.sqrt(n2)
    v = n / (1.0 + n2) * s_j
    return np.ascontiguousarray(v.astype(np.float32))


# revision 18
# speedup vs baseline: 1.1016x; 1.0025x over previous
"""CapsuleLayer (dynamic routing, 3 iters) on 8 TRN2 NeuronCores.

Strategy: shard the num_routes axis S=512 into 64 s-values per core.

Phase 1 (DMA-bound, ~100us): u_hat[b,s_loc,(o,c)] = x[b,s,:] @ W[s][:, (o,c)]
  streamed from HBM (W is 32 MiB/core in f16 — f16 is required: the routing
  softmax acts as a near-argmax over logits of magnitude ~1e2, so fp8 u_hat
  error (~2%) flips routing decisions and blows the 2e-2 gate).
  lhsT x-operands are zero-padded block-diagonal (XKZ[j] has x for s=4g+j in
  columns 32j..32j+32, zeros elsewhere) so the 4 s-values of a group
  accumulate into ONE [128, CO] PSUM tile -> one full-width ACT copy per
  group into U (f16, layout [p, g, o, c] — c innermost enables the DVE
  16-bit 2x mode for the weighted-sum multiply in phase 2).

Phase 2 (routing): partition p = 32*j + b.
  - agreement: fused MUL_CUMSUM over the (c,o) stream per g (f16 in, f32
    cumsum out), per-capsule sums recovered by differencing at o=63.
  - softmax over capsules: local (c on the free axis).
  - weighted sum: P = c_ij * u_hat as an all-f16 packed TensorTensor (DVE 2x
    mode), then the sum over (j, g) runs on the idle PE: 0/1 fold matrix
    FLD as lhsT, PSUM-accumulating over g. Only s_j = [B,CO] crosses cores:
    one AllReduce (two capsule halves) per routing iteration, 4 total; the
    final iteration emits per-core partials reduced on the host.
"""
import numpy as np

import concourse.bass as bass
import concourse.mybir as mybir
import concourse.tile as tile
from concourse import bacc
from concourse.bass_utils import run_bass_kernel_spmd
from concourse.masks import make_identity

B, S, C, DIN, DOUT = 32, 512, 16, 256, 64
NCORES = 8
S_LOC = S // NCORES          # 64
NG = S_LOC // 4              # 16 groups of 4 s-values
CO = C * DOUT                # 1024
CH = C // 2                  # capsules per half
KI = DIN // 128              # 2 contraction chunks
F32 = mybir.dt.float32
F32R = mybir.dt.float32r
F16 = mybir.dt.float16
AX = mybir.AxisListType
ALU = mybir.AluOpType
ACTF = mybir.ActivationFunctionType

_CACHE = {}


def _register_mul_cumsum():
    """out[p, :] = running cumsum of in0*in1 along the free stream.

    Registered at runtime (dve_ops.py is read-only here); same mechanism as
    the production ops — the per-NEFF DVE table is generated from OPS by
    name at compile time."""
    from concourse import dve_ops
    from concourse.dve_spec import Spec, Src0, Src1, AluOp, scan, lower as dve_lower
    from concourse.dve_uop import DveOpSpec

    name = "MUL_CUMSUM_ANT"
    for op in dve_ops.OPS:
        if op.name == name:
            return op

    def _ref(in0, in1, s0, s1, imm2):
        prod = (np.asarray(in0, np.float32) * np.asarray(in1, np.float32)).astype(
            np.float32
        )
        flat = prod.reshape(prod.shape[0], -1)
        return np.cumsum(flat, axis=1, dtype=np.float32).reshape(prod.shape)

    spec = Spec(body=scan(AluOp.ADD, Src0 * Src1), reference=_ref)
    row = dve_ops._CUSTOM_DVE_ROW_BASE + len(dve_ops.OPS)
    assert row < 0x20
    dve_ops._SUB_OPCODE_FOR_NAME[name] = row
    shas = {}
    for ver in ("v3", "v4"):
        uops = dve_lower(spec, ver=ver)
        shas[ver] = DveOpSpec(name=name, opcode=row, uops=uops, rd1_en=True).sha(ver)
    op = dve_ops.DveOp(name, spec, subdim=False, uops_sha=shas)
    dve_ops.OPS.append(op)
    dve_ops.CUSTOM_DVE_SPECS[name] = spec
    return op


MUL_CUMSUM = _register_mul_cumsum()


def _patch_act_tables():
    """Steer the act-table chooser to the ln+exp+copy set so squash (Ln,
    Exp) and softmax (Exp) share one table: without this the first-match rule
    alternates exp_and_others/natural_log, costing a 1.3us table load per
    switch. The combined set is a real act_info.json entry, so the emitted
    set id loads a table that genuinely contains every function used."""
    import concourse.bacc as _bacc_mod
    if getattr(_bacc_mod, "_ant_act_patched", False):
        return
    _orig = _bacc_mod.get_activation_tables

    def _patched(arch):
        out = {}
        for name, funcs in _orig(arch).items():
            if name != "natural_log_exp_and_others":
                funcs = funcs - {ACTF.Exp, ACTF.Ln}
            out[name] = funcs
        return out

    _bacc_mod.get_activation_tables = _patched
    _bacc_mod._ant_act_patched = True


_patch_act_tables()


def _build(sim_local=False, skip_routing=False, wbufs=3, dma_spread=0):
    nc = bacc.Bacc("TRN2", target_bir_lowering=False, debug=False, num_devices=NCORES)
    # Host pre-transposed inputs (per-core shards):
    #   xT: [128, KI, S_LOC, B] (partition = din within ki-chunk)
    #   wT: [S_LOC, DIN, DOUT*C]  — note (o, c) column order
    xT_ext = nc.declare_dram_parameter("xT", [128, KI, S_LOC, B], F16, isOutput=False)
    wT_ext = nc.declare_dram_parameter("wT", [S_LOC, DIN, CO], F16, isOutput=False)
    out_ext = nc.declare_dram_parameter("out", [B, CO], F32, isOutput=True)

    cc_in = [nc.dram_tensor(f"cc_in{k}", [B, CO // 2], F32) for k in range(4)]
    cc_out = [
        nc.dram_tensor(f"cc_out{k}", [B, CO // 2], F32, addr_space="Shared")
        for k in range(4)
    ]
    groups = [list(range(NCORES))]

    with tile.TileContext(nc) as tc:
        with (
            tc.tile_pool(name="persist", bufs=1) as pp,
            tc.tile_pool(name="psumP", bufs=1, space="PSUM") as pspP,
        ):
            # ---------------- phase 1: u_hat ----------------
            U = pp.tile([128, NG, DOUT, C], F16)      # u_hat, 32 KiB/part
            # Zero-padded block-diagonal lhsT: XKZ[:, j, ki, g, 32j:32j+32]
            # holds x[b, 4g+j, (ki,p)]; other columns zero. The 8 (j, ki)
            # matmuls of a group then accumulate the group's 4 u_hat rowsets
            # into one PSUM tile.
            XKZ = pp.tile([128, 4, KI, NG, 128], F16)
            nc.vector.memset(XKZ[:, 0:2], 0)
            nc.gpsimd.memset(XKZ[:, 2:4], 0)
            for j in range(4):
                nc.sync.dma_start(
                    out=XKZ[:, j, :, :, 32 * j : 32 * (j + 1)],
                    in_=xT_ext[:, :, j::4, :],
                )
            FLD = pp.tile([128, 32], F16)  # fold matrix: FLD[k, b] = (k%32 == b)
            make_identity(nc, FLD[0:32, :])
            for r in range(1, 4):
                nc.scalar.copy(FLD[32 * r : 32 * (r + 1), :], FLD[0:32, :])
            # Warm the (single) activation table under the DMA shadow so the
            # 1.3us LoadActFuncSet doesn't land on the critical path at AR0.
            WRM = pp.tile([1, 1], F32)
            nc.vector.memset(WRM[:], 1.0)
            nc.scalar.activation(WRM[:], WRM[:], ACTF.Ln)
            ps0 = pspP.tile([32, CO], F32, tag="ps0", name="ps0")
            with (
                tc.tile_pool(name="wpool", bufs=wbufs) as wp,
                tc.tile_pool(name="psum", bufs=2, space="PSUM") as psp,
            ):
                for g in range(NG):
                    wtg = wp.tile([128, 4, KI, CO], F16, tag="wt")
                    nc.sync.dma_start(
                        out=wtg[:],
                        in_=wT_ext[4 * g : 4 * (g + 1)].rearrange(
                            "s (k p) n -> p s k n", p=128
                        ),
                    )
                    ps = psp.tile([128, CO], F32, tag="ps")
                    for nh in range(2):
                        for j in range(4):
                            for ki in range(KI):
                                nc.tensor.matmul(
                                    ps[:, 512 * nh : 512 * (nh + 1)],
                                    XKZ[:, j, ki, g, :],
                                    wtg[:, j, ki, 512 * nh : 512 * (nh + 1)],
                                    start=(j == 0 and ki == 0),
                                    stop=(j == 3 and ki == KI - 1),
                                    skip_group_check=True,
                                )
                    nc.scalar.copy(
                        U[:, g].rearrange("p o c -> p (o c)"), ps[:]
                    )
                    # iter-0 partial sum over local s: fold (j, g) on the PE
                    # as PSUM-accumulated FLD matmuls over U. Emitted as three
                    # closed batches over groups whose U copies are already
                    # done, so the scheduler can run the first two under the
                    # DMA shadow instead of piling 32 matmuls on the tail.
                    batch = {9: range(0, 8), 14: range(8, 13), 15: range(13, 16)}
                    for gf in batch.get(g, ()):
                        for nh in range(2):
                            # each batch is a closed accumulation run (the
                            # later ones start=False onto the live PSUM), so
                            # the scheduler can place the early batches under
                            # the DMA shadow instead of clumping all 32
                            # matmuls at the phase boundary
                            nc.tensor.matmul(
                                ps0[:, 512 * nh : 512 * (nh + 1)],
                                FLD[:],
                                U[:, gf].rearrange("p o c -> p (o c)")[
                                    :, 512 * nh : 512 * (nh + 1)
                                ],
                                start=(gf == 0),
                                stop=(gf in (7, 12, NG - 1)),
                                skip_group_check=True,
                            )

            with (
                tc.tile_pool(name="ppool", bufs=6) as ppl,
                tc.tile_pool(name="psum2", bufs=1, space="PSUM") as psp2,
            ):
                # ---------------- phase 2: routing ----------------
                T = pp.tile([128, NG, C, DOUT], F32)   # cumsum scratch (full U)
                BL = pp.tile([128, NG, C], F32)        # b_ij logits
                BI = pp.tile([128, NG, C], F32)        # agreement increment
                SM = pp.tile([128, NG, C], F32)        # softmax scratch
                CI = pp.tile([128, NG, C], F16)        # c_ij (f16 for 2x P-mul)
                Mx = pp.tile([128, NG], F32)
                Zs = pp.tile([128, NG], F32)
                Rz = pp.tile([128, NG], F32)
                VR = pp.tile([128, C, DOUT], F16)      # v_j (c-major)
                SPH = pp.tile([32, C, DOUT], F32)      # AR send staging (c-major)
                SJH = pp.tile([32, C, DOUT], F32)      # AR result (s_j, c-major)
                # squash scratch (partitions 0:32 = batch)
                N2 = pp.tile([32, C], F32)
                Ny = pp.tile([32, C], F32)
                Ry = pp.tile([32, C], F32)
                Nt = pp.tile([32, C], F32)
                Y2 = pp.tile([32, C], F32)
                Dn = pp.tile([32, C], F32)
                Rd = pp.tile([32, C], F32)
                Fs = pp.tile([32, C], F32)
                XS = pp.tile([32, C, DOUT], F32)

                def allreduce_h(k, h):
                    """AllReduce one capsule-half of SPH into SJH; halves run
                    on independent DMA queues (sync, scalar) so the two
                    send/reduce/land chains overlap, and squash of half 0
                    hides under half 1's chain."""
                    engs = [nc.sync, nc.scalar]
                    idx = 2 * k + h
                    cs = slice(CH * h, CH * (h + 1))
                    engs[h].dma_start(
                        out=cc_in[idx][:],
                        in_=SPH[:, cs].rearrange("p c o -> p (c o)"),
                    )
                    if sim_local:
                        # TimelineSim can't model collectives; stand-in DMA.
                        engs[h].dma_start(out=cc_out[idx][:], in_=cc_in[idx][:])
                    else:
                        nc.gpsimd.collective_compute(
                            "AllReduce", ALU.add,
                            replica_groups=groups,
                            ins=[cc_in[idx][:]],
                            outs=[cc_out[idx][:]],
                        )
                    engs[h].dma_start(
                        out=SJH[:, cs].rearrange("p c o -> p (c o)"),
                        in_=cc_out[idx][:],
                    )

                def squash_h(h):
                    """One capsule-half of SJH -> v_j into VR[0:32, half],
                    then replicate that half across the j partition blocks on
                    ACT (so DVE can squash the other half concurrently).
                    sqrt via exp(0.5*ln) + one Newton step."""
                    cs = slice(CH * h, CH * (h + 1))
                    sj = SJH[:, cs]
                    n2, ny, ry, nt = N2[:, cs], Ny[:, cs], Ry[:, cs], Nt[:, cs]
                    y2, dn, rd, fs = Y2[:, cs], Dn[:, cs], Rd[:, cs], Fs[:, cs]
                    nc.vector.tensor_mul(XS[:, cs], sj, sj)
                    nc.vector.tensor_reduce(n2, XS[:, cs], axis=AX.X, op=ALU.add)
                    nc.scalar.activation(ny, n2, ACTF.Ln)
                    nc.scalar.activation(ny, ny, ACTF.Exp, scale=0.5)
                    # Newton: y = 0.5*(y0 + n2/y0)
                    nc.vector.reciprocal(ry, ny)
                    nc.vector.tensor_mul(nt, n2, ry)
                    nc.vector.tensor_add(y2, ny, nt)
                    nc.vector.tensor_scalar_mul(y2, y2, 0.5)
                    # f = y / (1 + n2);  v = s_j * f
                    nc.vector.tensor_scalar_add(dn, n2, 1.0)
                    nc.vector.reciprocal(rd, dn)
                    nc.vector.tensor_mul(fs, y2, rd)
                    nc.vector.tensor_mul(
                        VR[0:32, cs], sj, fs.broadcast_to([32, CH, DOUT])
                    )

                def replicate_v():
                    # replicate v across the 3 other j partition blocks,
                    # full-width, split across ACT and DVE
                    nc.scalar.copy(VR[32:64], VR[0:32])
                    nc.vector.tensor_copy(VR[64:96], VR[0:32])
                    nc.vector.tensor_copy(VR[96:128], VR[0:32])

                def agreement(first):
                    """dst[:, g, c] = sum_o u_hat*v via ONE fused mul-cumsum
                    over the whole (g, c, o) stream; per-(g,c) sums recovered
                    by differencing the f32 cumsum at the o=63 positions.
                    (Adjacent cum values share their long prefix, so the
                    differencing error is only the within-segment rounding,
                    ~sqrt(64)*ulp — negligible in f32.)"""
                    dst = BL if first else BI
                    for g in range(NG):
                        nc.vector._custom_dve(
                            MUL_CUMSUM,
                            out=T[:, g].rearrange("p c o -> p (c o)"),
                            in0=U[:, g].rearrange("p o c -> p c o"),
                            in1=VR[:],
                        )
                    cum63 = T[:, :, :, DOUT - 1]  # [p, g, c]
                    nc.vector.tensor_copy(dst[:, :, 0:1], cum63[:, :, 0:1])
                    nc.vector.tensor_sub(
                        dst[:, :, 1:], cum63[:, :, 1:], cum63[:, :, 0 : C - 1]
                    )

                def softmax(from_bl_plus_bi):
                    src = BL
                    if from_bl_plus_bi:
                        nc.vector.tensor_add(BL[:], BL[:], BI[:])
                    nc.vector.tensor_reduce(Mx[:], src[:], axis=AX.X, op=ALU.max)
                    nc.vector.tensor_sub(SM[:], src[:], Mx[:].broadcast_to([128, NG, C]))
                    nc.scalar.activation(CI[:], SM[:], ACTF.Exp)
                    nc.vector.tensor_reduce(Zs[:], CI[:], axis=AX.X, op=ALU.add)
                    nc.vector.reciprocal(Rz[:], Zs[:])
                    nc.vector.tensor_mul(CI[:], CI[:], Rz[:].broadcast_to([128, NG, C]))

                def pe_warmup(tag):
                    """~3us of dummy matmuls (overlapping softmax, PE idle
                    otherwise) so the PE p-state is fully ramped when the
                    weighted-sum fold matmuls arrive."""
                    psd = psp2.tile([32, 512], F32, tag=tag, name=tag)
                    for i in range(14):
                        nc.tensor.matmul(
                            psd[:],
                            FLD[:],
                            U[:, 0].rearrange("p o c -> p (o c)")[:, 0:512],
                            start=(i == 0),
                            stop=(i == 13),
                            skip_group_check=True,
                        )

                def weighted_sum(tag):
                    """psum[b, (o,c)] = sum_{j,g} c_ij*u_hat: per-g f16 2x-mode
                    multiply on DVE, then the (j, g)-fold runs on the idle PE
                    (FLD as lhsT, PSUM accumulation over g)."""
                    psw = psp2.tile([32, CO], F32, tag=tag, name=tag)
                    for g in range(NG):
                        Pt = ppl.tile([128, DOUT, C], F16, tag="P")
                        nc.vector.tensor_mul(
                            Pt[:],
                            U[:, g],
                            CI[:, g, :]
                            .broadcast_to([128, C, DOUT])
                            .rearrange("p c o -> p o c"),
                        )
                        for nh in range(2):
                            nc.tensor.matmul(
                                psw[:, 512 * nh : 512 * (nh + 1)],
                                FLD[:],
                                Pt[:].rearrange("p o c -> p (o c)")[
                                    :, 512 * nh : 512 * (nh + 1)
                                ],
                                start=(g == 0),
                                stop=(g == NG - 1),
                            )
                    return psw

                if skip_routing:
                    nc.scalar.copy(SPH[:], ps0[:])
                    nc.sync.dma_start(out=out_ext[:], in_=SPH[:])
                    nc.compile()
                    return nc

                def stage_and_reduce(k, psrc, scale):
                    """PSUM -> SPH (c-major transpose copy) per half, AR both
                    halves on separate queues, squash each half as it lands."""
                    pv = psrc[:].rearrange("b (o c) -> b o c", o=DOUT)
                    for h in range(2):
                        cs = slice(CH * h, CH * (h + 1))
                        if scale is None:
                            nc.scalar.copy(
                                SPH[:, cs], pv[:, :, cs].rearrange("b o c -> b c o")
                            )
                        else:
                            nc.scalar.mul(
                                SPH[:, cs],
                                pv[:, :, cs].rearrange("b o c -> b c o"),
                                scale,
                            )
                        allreduce_h(k, h)
                    for h in range(2):
                        squash_h(h)
                    replicate_v()

                # ---- iter 0: c_ij uniform = 1/C ----
                stage_and_reduce(0, ps0, 1.0 / C)
                agreement(first=True)

                # ---- iter 1 ----
                pe_warmup("wu1")
                softmax(from_bl_plus_bi=False)
                psw = weighted_sum("psw1")
                stage_and_reduce(1, psw, None)
                agreement(first=False)

                # ---- iter 2 ----
                # The final cross-core reduce + squash are part of the host
                # unshard: each core emits its local sum_s c_ij*u_hat partial.
                pe_warmup("wu2")
                softmax(from_bl_plus_bi=True)
                psw2 = weighted_sum("psw2")
                OUTS = pp.tile([32, CO], F32)
                engs = [nc.sync, nc.scalar]
                for h in range(2):
                    cols = slice(512 * h, 512 * (h + 1))
                    nc.scalar.copy(OUTS[:, cols], psw2[:, cols])
                    engs[h].dma_start(out=out_ext[:, cols], in_=OUTS[:, cols])

    nc.compile()
    return nc


def _get_nc():
    if "nc" not in _CACHE:
        _CACHE["nc"] = _build()
    return _CACHE["nc"]


def _get_runner():
    """Cached shard_map executable over the 8 cores (mirrors
    bass2jax.run_bass_via_pjrt, but reusable across calls and without the
    per-core concat — the s-outer host layout makes the global concatenated
    input exactly xT/wT)."""
    if "runner" in _CACHE:
        return _CACHE["runner"]
    import jax
    from jax.sharding import Mesh, PartitionSpec
    from jax.experimental.shard_map import shard_map
    from concourse import bass2jax as b2j

    nc = _get_nc()
    b2j.install_neuronx_cc_hook()
    partition_name = nc.partition_id_tensor.name if nc.partition_id_tensor else None
    in_names, out_names, out_avals = [], [], []
    for alloc in nc.m.functions[0].allocations:
        if not isinstance(alloc, mybir.MemoryLocationSet):
            continue
        name = alloc.memorylocations[0].name
        if alloc.kind == "ExternalInput":
            if name != partition_name:
                in_names.append(name)
        elif alloc.kind == "ExternalOutput":
            out_names.append(name)
            out_avals.append(
                jax.core.ShapedArray(tuple(alloc.tensor_shape), mybir.dt.np(alloc.dtype))
            )
    n_params = len(in_names)
    all_in_names = list(in_names) + list(out_names)
    if partition_name is not None:
        all_in_names.append(partition_name)

    def _body(*args):
        operands = list(args)
        if partition_name is not None:
            operands.append(b2j.partition_id_tensor())
        outs = b2j._bass_exec_p.bind(
            *operands,
            out_avals=tuple(out_avals),
            in_names=tuple(all_in_names),
            out_names=tuple(out_names),
            lowering_input_output_aliases=(),
            sim_require_finite=True,
            sim_require_nnan=True,
            nc=nc,
        )
        return tuple(outs)

    devices = jax.devices()[:NCORES]
    mesh = Mesh(np.asarray(devices), ("core",))
    n_outs = len(out_names)
    sharded = jax.jit(
        shard_map(
            _body,
            mesh=mesh,
            in_specs=(PartitionSpec("core"),) * (n_params + n_outs),
            out_specs=(PartitionSpec("core"),) * n_outs,
            check_rep=False,
        ),
        donate_argnums=tuple(range(n_params, n_params + n_outs)),
        keep_unused=True,
    )
    _CACHE["runner"] = (sharded, in_names, out_names, out_avals)
    return _CACHE["runner"]


def kernel(x: np.ndarray, W: np.ndarray) -> np.ndarray:
    assert x.shape == (B, S, DIN) and W.shape == (C, S, DIN, DOUT)
    xf = x.astype(np.float32)
    xk = np.empty((NCORES * 128, KI, S_LOC, B), np.float16)
    for c in range(NCORES):
        sl = xf[:, c * S_LOC : (c + 1) * S_LOC, :]  # [B, S_LOC, DIN]
        for ki in range(KI):
            xk[c * 128 : (c + 1) * 128, ki] = sl[
                :, :, ki * 128 : (ki + 1) * 128
            ].transpose(2, 1, 0)
    # W columns in (o, c) order: wT[s, i, (o, c)]
    wT = np.ascontiguousarray(
        np.transpose(W.astype(np.float32), (1, 2, 3, 0)).reshape(S, DIN, CO)
    ).astype(np.float16)
    sharded, in_names, out_names, out_avals = _get_runner()
    ins = {"xT": xk, "wT": wT}
    concat_in = [ins[name] for name in in_names]
    concat_zeros = [
        np.zeros((NCORES * a.shape[0], *a.shape[1:]), a.dtype) for a in out_avals
    ]
    out_arrs = sharded(*concat_in, *concat_zeros)
    parts = np.asarray(out_arrs[out_names.index("out")]).reshape(
        NCORES, B, DOUT, C
    )
    s_j = parts.astype(np.float64).sum(axis=0).transpose(0, 2, 1)  # [B, C, DOUT]
    n2 = np.sum(s_j * s_j, axis=-1, keepdims=True)
    n = np.sqrt(n2)
    v = n / (1.0 + n2) * s_j
    return np.ascontiguousarray(v.astype(np.float32))
